# revision 52
# baseline (speedup 1.0000x reference)
"""Trainium2 Bass kernel: multi-head attention (b=4, s=2048, d_model=1024, h=16).

Sharding over 8 NeuronCores: 2-D (batch x head-half).
  core c -> batch c//2, head group c%2 (8 of 16 heads, qkv dims 512*g..512*g+512).
Per core: QKV column-parallel, per-head attention (scores computed transposed,
softmax sums via a ones-column appended to V in the PV matmul, max-subtraction
skipped -- scores are O(5) so exp is safe), then a pairwise AllGather of the
normalized per-head outputs and a column-parallel output projection.

All matmul operands are bf16 (fp32 PSUM accumulation); fp32 matmul on trn2
costs two array passes, bf16 one. The host pre-transposes x to x^T [D, S]
and casts to bf16 (input prep), so no on-device transpose is needed.

QKV projection (per head-pair) and attention are emitted interleaved so the
scalar-engine exp stream (the critical resource) starts as early as possible
and PE fills its gaps with the next head-pair's projections. The AllGather
is split per head-pair so all but the last overlap attention.

v2 changes (from trace analysis of the 421us baseline):
 - PE warm-up matmul stream at t=0 so the HAM clock gate reaches 2.4 GHz
   before real matmuls arrive (PE ran at 1.2 GHz until 35us).
 - Consolidated input loads into one fat DMA per tensor (DMA issue on the
   sync queue costs ~590ns each; 24 thin descriptors gated the first
   matmul at 17.9us).
 - Softmax extract/normalize rebuilt: acc PSUM released after two fp32
   copies, reciprocal via reciprocal_approx_fast (5x cheaper than the
   3.3us RECIPROCAL that stalled PV between units), fp32 broadcast (no
   bf16 cast hop), normalize fused into the evacuation multiply. Per-unit
   small DMAs moved to the gpsimd queue.
 - Tail: hp2/hp3 gathered in 512-col chunks, staging DMAs are 2 fat
   descriptors, and the final Wo phase-B is emitted per 512-token quarter
   right after its staging flush so the endgame pipelines instead of
   serializing (~40us tail before).

Host assembly: out[b] = concat(core 2b cols 0:512, core 2b+1 cols 512:1024).

Self-contained: hardcodes all shapes; builds/compiles once per process.
"""

from contextlib import ExitStack

import ml_dtypes
import numpy as np

import concourse.bass as bass
import concourse.mybir as mybir
import concourse.tile as tile
from concourse import bacc
from concourse.bass_utils import run_bass_kernel_spmd

FP = mybir.dt.float32
FR = mybir.dt.float32r
BF = mybir.dt.bfloat16
AFT = mybir.ActivationFunctionType
ts = bass.ts

NCORES = 8
D = 1024           # d_model
HD = 64            # head dim
HPC = 8            # heads per core
DQ = HPC * HD      # per-core qkv width = 512
SCALE = 1.0 / np.sqrt(HD)
N_WARMUP = 8       # PE warm-up matmuls: ~4.5us cold, bridges to data-ready


def emit_mha(nc, tc, io, S, dbg=False):
    """Emit the per-core MHA program. io: dict of DRAM APs."""
    NHP = HPC // 2       # head pairs = 4
    KT = S // 128        # sk tiles
    SQB = S // 512       # sq blocks of 512
    DKT = D // 128       # d_in tiles = 8
    MQ = DQ // 128       # qkv dout tiles = 4
    TT = S // 128        # token tiles
    NB = S // 512        # token blocks of 512

    xt_in, wq_in, bqk_in, wk_in, wv_in, bv_in, wo_in, bo_in, out_ext = (
        io["xt"], io["wq"], io["bqk"], io["wk"], io["wv"], io["bv"],
        io["wo"], io["bo"], io["out"])

    with ExitStack() as ctx:
        const_pool = ctx.enter_context(tc.tile_pool(name="const", bufs=1))
        dram_pool = ctx.enter_context(tc.tile_pool(name="dram", bufs=1, space="DRAM"))
        # one shared PSUM budget: mm 2 + scores 4 + accA 1 + accB 1 = 8 banks
        mm_psum = ctx.enter_context(
            tc.tile_pool(name="mmps", bufs=2, space="PSUM"))
        sc_psum = ctx.enter_context(
            tc.tile_pool(name="scps", bufs=2, space="PSUM"))
        ac_psum = ctx.enter_context(
            tc.tile_pool(name="acps", bufs=1, space="PSUM"))

        # ---- PE warm-up: HAM-unthrottle the tensor engine while the input
        # DMAs are in flight. Reuses the mm psum tag (no extra banks). ----
        warm = const_pool.tile([128, 512], BF, tag="warm", name="warm")
        nc.gpsimd.memset(warm[:], 1.0)
        for _ in range(N_WARMUP):
            wp = mm_psum.tile([128, 512], FP, tag="mm", name="mm")
            nc.tensor.matmul(wp[:], lhsT=warm[:, 0:128],
                             rhs=warm[:], start=True, stop=True)

        # biases for q/k, host-packed [128, 2*MQ]: col m = bq tile m, MQ+m = bk
        bias_qk = const_pool.tile([128, 2 * MQ], FP, tag="bqk", name="bqk")
        nc.sync.dma_start(bias_qk[:], bqk_in[:, :])



        # bv / bo broadcast tiles; bo twice along the free dim since phase A
        # adds it over [128, 1024] psum pairs
        bv_bc = const_pool.tile([128, DQ], FP, tag="bvbc", name="bvbc")
        bo_bc2 = const_pool.tile([128, 2 * DQ], FP, tag="bobc", name="bobc")
        with tc.tile_pool(name="btmpp", bufs=1) as btmp_pool:
            btmp = btmp_pool.tile([128, DQ], FP, tag="btmp", name="btmp")
            nc.sync.dma_start(
                btmp[0:1, :], bv_in[:].rearrange("(one f) -> one f", one=1))
            nc.gpsimd.partition_broadcast(bv_bc[:], btmp[0:1, :])
            btmp2 = btmp_pool.tile([128, 2 * DQ], FP, tag="btmp2",
                                   name="btmp2")
            for h in range(2):
                nc.sync.dma_start(
                    btmp2[0:1, ts(h, DQ)],
                    bo_in[:].rearrange("(one f) -> one f", one=1))
            nc.gpsimd.partition_broadcast(bo_bc2[:], btmp2[0:1, :])

        # DRAM bounce + AllGather in/out (bf16); collective operands must be
        # contiguous. Early head-pairs gather per seq-half; the last two
        # gather in 512-col chunks so the endgame only ever waits on 512
        # columns and the final Wo accumulation pipelines per quarter.
        def seq_chunks(hp):
            if hp < 2:
                return [(1, 0, S // 2), (3, S // 2, S // 2)]
            if hp == 2:
                return [(1, 0, S // 2), (2, S // 2, 512), (3, S // 2 + 512, 512)]
            return [(0, 0, 512), (1, 512, 512), (2, 1024, 512), (3, 1536, 512)]

        y_bnc = {}
        y_gath = {}
        for hp in range(NHP):
            for (sd, c0, w) in seq_chunks(hp):
                y_bnc[hp, c0] = dram_pool.tile(
                    [128, w], BF, tag=f"ybounce{hp}_{c0}",
                    name=f"ybounce{hp}_{c0}")
                y_gath[hp, c0] = dram_pool.tile(
                    [256, w], BF, tag=f"ygather{hp}_{c0}",
                    name=f"ygather{hp}_{c0}")

        with ExitStack() as phase12:
            qkv_pool = phase12.enter_context(tc.tile_pool(name="qkv", bufs=1))
            yt_pool = phase12.enter_context(tc.tile_pool(name="yt", bufs=1))
            exp_pool = phase12.enter_context(tc.tile_pool(name="exp", bufs=6))
            stage_pool = phase12.enter_context(tc.tile_pool(name="stage", bufs=2))

            # q^T / k^T, d-major: tile hp holds heads 2hp (parts 0-63), 2hp+1
            qT = [qkv_pool.tile([128, S], BF, tag=f"qT{m}", name=f"qT{m}")
                  for m in range(MQ)]
            kT = [qkv_pool.tile([128, S], BF, tag=f"kT{m}", name=f"kT{m}")
                  for m in range(MQ)]
            # v natural [tok, dout] with a ones column per head
            v_ones = [qkv_pool.tile([128, HPC * (HD + 1)], BF, tag=f"v{t}",
                                    name=f"v{t}")
                      for t in range(TT)]

            # Wo + gathered-y staging (loaded during attention so the
            # final projection phase starts compute immediately)
            yg = [qkv_pool.tile([128, S], BF, tag=f"yg{k}", name=f"yg{k}")
                  for k in range(2 * MQ)]

            def stage_yg(hp, c0, w, eng=None):
                # copy gathered chunk into the staging tiles (2 fat DMAs).
                # In-loop flushes (wait-free, gather long done) ride the
                # fast sync DGE. The post-loop, gather-waiting flushes are
                # issued from the scalar queue instead: sync-queue DMAs
                # share round-robin completion semaphores, so a
                # gather-waiting stage DMA there poisons the wait of the
                # next collective trigger (observed +10us on the last
                # gather).
                eng = eng or nc.sync
                gath = y_gath[hp, c0]
                eng.dma_start(yg[hp][:, c0:c0 + w], gath[0:128, :])
                eng.dma_start(yg[MQ + hp][:, c0:c0 + w], gath[128:256, :])

            with ExitStack() as phase01:
                # ---- load x^T (pre-transposed on host) and weights; one
                # fat DMA per tensor (issue rate, not bandwidth, gated the
                # kernel head) ----
                xtw_pool = phase01.enter_context(tc.tile_pool(name="xtw", bufs=1))
                xTall = xtw_pool.tile([128, DKT * S], BF, tag="xTall",
                                      name="xTall")
                xT3 = xTall[:].rearrange("p (d s) -> p d s", s=S)
                xT4 = xTall[:].rearrange("p (d nb s) -> p d nb s", nb=NB,
                                         s=512)
                xt_src = xt_in.rearrange("(k nb p) s -> p k nb s", k=DKT,
                                         nb=NB, p=128)

                def xTs(k, sl):
                    return xT3[:, k, sl]

                def load_x_nb(nb, k0=0, k1=DKT):
                    nc.sync.dma_start(xT4[:, k0:k1, nb, :],
                                      xt_src[:, k0:k1, nb, :])

                # weights as single tiles, k-major free layout:
                # w_all[p, k*DQ + c] = W[k*128 + p, c]. All bulk loads ride
                # the sync DGE (the only ~155GB/s path; scalar/gpsimd DGEs
                # measured far slower), ordered by first consumption: the
                # m=0 column chunks of wq/wk land first so head-pair 0's
                # projections (and the exp stream) start ~14us in.
                def walloc(nm):
                    return xtw_pool.tile([128, DKT * DQ], BF, tag=nm,
                                         name=nm)

                def load_w_cols(t, w_in, c0, c1):
                    nc.sync.dma_start(
                        t[:].rearrange("p (k c) -> p k c", c=DQ)[:, :, c0:c1],
                        w_in.rearrange("(k p) c -> p k c", p=128)[:, :, c0:c1])

                wq_all, wk_all, wv_all, wo_all = (
                    walloc("wqa"), walloc("wka"), walloc("wva"), walloc("woa"))
                load_x_nb(0, 0, 4)
                load_w_cols(wq_all, wq_in, 0, 128)
                load_w_cols(wk_all, wk_in, 0, 128)
                load_x_nb(0, 4, 8)
                load_w_cols(wv_all, wv_in, 0, DQ)
                load_x_nb(1)
                load_w_cols(wq_all, wq_in, 128, DQ)
                load_w_cols(wk_all, wk_in, 128, DQ)
                load_w_cols(wo_all, wo_in, 0, DQ)
                load_x_nb(2)
                load_x_nb(3)

                def emit_qk_group(m, g):
                    # one q/k projection psum group for head-pair m;
                    # g//NB selects q vs k, g%NB the token block. Evac on
                    # DVE (keeps the scalar engine free for the exp stream).
                    w_all, bcol, dstT = ((wq_all, 0, qT), (wk_all, 1, kT))[g // NB]
                    nb = g % NB
                    ps = mm_psum.tile([128, 512], FP, tag="mm", name="mm")
                    for k in range(DKT):
                        nc.tensor.matmul(
                            ps[:], lhsT=w_all[:, k * DQ + m * 128:
                                              k * DQ + (m + 1) * 128],
                            rhs=xTs(k, ts(nb, 512)),
                            start=(k == 0), stop=(k == DKT - 1))
                    col = bcol * MQ + m
                    nc.vector.tensor_scalar_add(
                        dstT[m][:, ts(nb, 512)], ps[:],
                        bias_qk[:, col:col + 1])

                def emit_qk_pair(m, ga, gb):
                    # two same-family projection groups emitted with the
                    # k-loop interleaved: consecutive matmuls share their
                    # stationary weight block, letting the PE skip/overlap
                    # the second LDWEIGHTS.
                    w_all, bcol, dstT = ((wq_all, 0, qT), (wk_all, 1, kT))[ga // NB]
                    assert gb // NB == ga // NB
                    nba, nbb = ga % NB, gb % NB
                    psa = mm_psum.tile([128, 512], FP, tag="mm", name="mm")
                    psb = mm_psum.tile([128, 512], FP, tag="mm", name="mm")
                    for k in range(DKT):
                        wsl = w_all[:, k * DQ + m * 128: k * DQ + (m + 1) * 128]
                        nc.tensor.matmul(
                            psa[:], lhsT=wsl, rhs=xTs(k, ts(nba, 512)),
                            start=(k == 0), stop=(k == DKT - 1),
                            skip_group_check=True)
                        nc.tensor.matmul(
                            psb[:], lhsT=wsl, rhs=xTs(k, ts(nbb, 512)),
                            start=(k == 0), stop=(k == DKT - 1),
                            skip_group_check=True)
                    col = bcol * MQ + m
                    nc.vector.tensor_scalar_add(
                        dstT[m][:, ts(nba, 512)], psa[:],
                        bias_qk[:, col:col + 1])
                    nc.vector.tensor_scalar_add(
                        dstT[m][:, ts(nbb, 512)], psb[:],
                        bias_qk[:, col:col + 1])

                def emit_v(t0, t1):
                    for ti in range(t0, t1):
                        ps = mm_psum.tile([128, DQ], FP, tag="mm", name="mm")
                        for k in range(DKT):
                            nc.tensor.matmul(
                                ps[:], lhsT=xTs(k, ts(ti, 128)),
                                rhs=wv_all[:, ts(k, DQ)],
                                start=(k == 0), stop=(k == DKT - 1))
                        vt3 = v_ones[ti][:].rearrange("p (h u) -> p h u",
                                                      u=HD + 1)
                        nc.vector.tensor_add(
                            vt3[:, :, 0:HD],
                            ps[:].rearrange("p (h u) -> p h u", u=HD),
                            bv_bc[:].rearrange("p (h u) -> p h u", u=HD))
                        nc.gpsimd.memset(vt3[:, :, HD:HD + 1], 1.0)

                # ---- per head-pair: qk projection then attention; hp0's
                # q/k groups interleaved into the first unit so the exp
                # stream starts asap; gathered-y staging DMAs deferred 2
                # units so they never wait on an in-flight collective ----
                pending_stage = []

                def flush_stage(unit_now):
                    while pending_stage and pending_stage[0][3] <= unit_now - 2:
                        rhp, rc0, rw, _ = pending_stage.pop(0)
                        stage_yg(rhp, rc0, rw)

                for hp in range(NHP):
                    if hp == 0:
                        emit_qk_group(0, 0)       # q nb0
                        emit_qk_group(0, NB)      # k nb0

                    # y^T for this head-pair (d-major, normalized in place
                    # per unit); ring of 2 so hp+1 overlaps hp's shipping
                    yTh = yt_pool.tile([128, S], BF, tag="yT", bufs=2,
                                       name="yT")
                    hA, hB = 2 * hp, 2 * hp + 1
                    for sqb in range(SQB):
                        if hp == 0 and sqb > 0:
                            emit_qk_group(0, sqb)  # q nb<sqb>
                        sq = ts(sqb, 512)
                        accA = ac_psum.tile([HD + 1, 512], FP, tag="accA",
                                            name="accA")
                        accB = ac_psum.tile([HD + 1, 512], FP, tag="accB",
                                            name="accB")
                        for k in range(KT):
                            if hp == 0 and sqb == 0 and k % 4 == 0 and k > 0:
                                emit_qk_group(0, NB + k // 4)  # k nb<k//4>
                            sk = ts(k, 128)
                            ps = sc_psum.tile([128, 1024], FP, tag="sc", name="sc")
                            # scores^T [sk, sq] for both heads (row-tiled pair)
                            nc.tensor.matmul(
                                ps[:, 0:512], lhsT=kT[hp][0:64, sk],
                                rhs=qT[hp][0:64, sq], start=True, stop=True)
                            nc.tensor.matmul(
                                ps[:, 512:1024], lhsT=kT[hp][64:128, sk],
                                rhs=qT[hp][64:128, sq], start=True, stop=True)
                            et = exp_pool.tile([128, 1024], BF, tag="exp",
                                               name="exp")
                            nc.scalar.activation(et[:], ps[:], AFT.Exp,
                                                 scale=SCALE)
                            if hp == 0 and sqb == 0:
                                # produce v[k] just in time for its attnv
                                emit_v(k, k + 1)
                            # y^T accumulation: lhsT = [v_h | 1]
                            nc.tensor.matmul(
                                accA[:], lhsT=v_ones[k][:, hA * 65:hA * 65 + 65],
                                rhs=et[:, 0:512],
                                start=(k == 0), stop=(k == KT - 1),
                                skip_group_check=True)
                            nc.tensor.matmul(
                                accB[:], lhsT=v_ones[k][:, hB * 65:hB * 65 + 65],
                                rhs=et[:, 512:1024],
                                start=(k == 0), stop=(k == KT - 1),
                                skip_group_check=True)
                        # ---- extract + normalize ----
                        # Two fp32 copies release the acc PSUM fast; the
                        # sums rows hop to partition 0 (sync DMA, short
                        # wait), feed reciprocal_approx_fast (sums are
                        # O(1e2..1e5), far from its edge cases), an fp32
                        # partition broadcast, and a normalize multiply
                        # fused with the evacuation. Queue placement breaks
                        # the cross-queue cycle seen in traces: sync holds
                        # only short-wait hops + the ship, while the hop
                        # that waits on the multiply rides gpsimd ahead of
                        # the collective trigger it feeds.
                        stA = stage_pool.tile([128, 512], FP, tag="stA",
                                              name="stA")
                        stB = stage_pool.tile([128, 512], FP, tag="stB",
                                              name="stB")
                        nc.vector.tensor_copy(stA[0:65, :], accA[0:65, :])
                        nc.vector.tensor_copy(stB[0:65, :], accB[0:65, :])
                        bbA = stage_pool.tile([1, 512], FP, tag="bbA",
                                              name="bbA")
                        bbB = stage_pool.tile([1, 512], FP, tag="bbB",
                                              name="bbB")
                        nc.sync.dma_start(bbA[0:1, :], stA[64:65, :])
                        nc.sync.dma_start(bbB[0:1, :], stB[64:65, :])
                        nc.vector.reciprocal_approx_fast(bbA[0:1, :],
                                                         bbA[0:1, :])
                        nc.vector.reciprocal_approx_fast(bbB[0:1, :],
                                                         bbB[0:1, :])
                        rbA = stage_pool.tile([128, 512], FP, tag="rbA",
                                              name="rbA")
                        rbB = stage_pool.tile([128, 512], FP, tag="rbB",
                                              name="rbB")
                        nc.gpsimd.partition_broadcast(rbA[0:64, :], bbA[0:1, :])
                        nc.gpsimd.partition_broadcast(rbB[0:64, :], bbB[0:1, :])
                        nc.vector.tensor_mul(yTh[0:64, sq], stA[0:64, :],
                                             rbA[0:64, :])
                        stB16 = stage_pool.tile([128, 512], BF, tag="stB16",
                                                name="stB16")
                        nc.vector.tensor_mul(stB16[0:64, :], stB[0:64, :],
                                             rbB[0:64, :])
                        nc.gpsimd.dma_start(yTh[64:128, sq], stB16[0:64, :])
                        # drip the next head-pair's projections between units
                        # (paired: both groups of a unit share weight blocks)
                        if hp + 1 < NHP:
                            emit_qk_pair(hp + 1, 2 * sqb, 2 * sqb + 1)
                        # ship and AllGather each completed seq chunk of this
                        # hp's y^T; defer the gathered-y staging DMAs
                        for (sd, c0, w) in seq_chunks(hp):
                            if sd != sqb:
                                continue
                            nc.sync.dma_start(y_bnc[hp, c0][:, :],
                                              yTh[:, c0:c0 + w])
                            nc.gpsimd.collective_compute(
                                "AllGather", mybir.AluOpType.bypass,
                                replica_groups=[[0, 1], [2, 3],
                                                [4, 5], [6, 7]],
                                ins=[y_bnc[hp, c0][:, :]],
                                outs=[y_gath[hp, c0][:, :]])
                            pending_stage.append(
                                (hp, c0, w, hp * SQB + sqb))
                        flush_stage(hp * SQB + sqb)
                # ---- Phase A of the output projection: per token tile,
                # accumulate head-pair-0..2 contributions (6 of 8 k2 blocks,
                # staged well before attention ends) + bias into a bf16
                # partial parked in a dead qT/kT tile. Emitted AFTER the hp
                # loop so its priority is lowest: the scheduler runs these
                # matmuls in PE-idle slots of hp3's exp-paced units. Only
                # tiles 0-11 (72 matmuls): that is what the real slack of
                # the exp-paced hp3 units holds. With all 16 tiles, the
                # overflow group lands after the whole attention stream and
                # its DVE add -- statically ordered mid-stream by the
                # scheduler's optimistic PE model -- head-of-line-blocks
                # the unit-14/15 extract ops for ~10us (semaphores are
                # counters over the static per-engine order). Tiles 12-15
                # run as full 8-band groups in phase B instead. ----
                for t0 in range(0, TT, 2):
                    pos = [mm_psum.tile([128, DQ], FP, tag="mm", name="mm")
                           for _ in range(2)]
                    for j, k2 in enumerate((0, 1, 2, 4, 5, 6)):
                        for po, t in zip(pos, (t0, t0 + 1)):
                            nc.tensor.matmul(
                                po[:], lhsT=yg[k2][:, ts(t, 128)],
                                rhs=wo_all[:, ts(k2, DQ)],
                                start=(j == 0), stop=(j == 5),
                                skip_group_check=True)
                    for po, t in zip(pos, (t0, t0 + 1)):
                        slot = (qT[:3] + [kT[0]])[t // 4]
                        nc.vector.tensor_add(
                            slot[:, ts(t % 4, 512)], po[:],
                            bo_bc2[:, 0:DQ])

                # ---- Phase B: add the head-pair-3 contribution and store,
                # pipelined per 512-token quarter behind its staging. ----
                with tc.tile_pool(name="outp", bufs=3) as out_pool:
                    for q in range(SQB):
                        while pending_stage and (
                                pending_stage[0][0] < NHP - 1
                                or pending_stage[0][1] <= 512 * q):
                            rhp, rc0, rw, _ = pending_stage.pop(0)
                            stage_yg(rhp, rc0, rw)
                        for t0 in (4 * q, 4 * q + 2):
                            pos = [mm_psum.tile([128, DQ], FP, tag="mm",
                                                name="mm")
                                   for _ in range(2)]
                            k2s = (NHP - 1, MQ + NHP - 1)
                            for j, k2 in enumerate(k2s):
                                for po, t in zip(pos, (t0, t0 + 1)):
                                    nc.tensor.matmul(
                                        po[:], lhsT=yg[k2][:, ts(t, 128)],
                                        rhs=wo_all[:, ts(k2, DQ)],
                                        start=(j == 0),
                                        stop=(j == len(k2s) - 1),
                                        skip_group_check=True)
                            for po, t in zip(pos, (t0, t0 + 1)):
                                ot = out_pool.tile([128, DQ], BF, tag="ot",
                                                   name="ot")
                                slot = (qT[:3] + [kT[0]])[t // 4]
                                nc.vector.tensor_add(
                                    ot[:], po[:], slot[:, ts(t % 4, 512)])
                                # bf16 stores (host upcasts; halves the
                                # store bytes) on the scalar queue (idle
                                # once the exp stream ends); they must not
                                # block sync (ships) or gpsimd (triggers +
                                # staging). The last quarter's stores ride
                                # the fast sync DGE: nothing queues behind
                                # them there.
                                eng = nc.sync if q == SQB - 1 else nc.scalar
                                eng.dma_start(out_ext[ts(t, 128), :], ot[:])


def build_program(S=2048, dbg=False):
    nc = bacc.Bacc(
        "TRN2",
        target_bir_lowering=False,
        debug=False,
        enable_asserts=True,
        num_devices=NCORES,
    )
    NB = S // 512
    DKT = D // 128
    io = {
        "xt": nc.declare_dram_parameter("xt", [DKT * NB * 128, 512], BF,
                                        isOutput=False),
        "wq": nc.declare_dram_parameter("wq", [D, DQ], BF, isOutput=False),
        "bqk": nc.declare_dram_parameter("bqk", [128, 8], FP, isOutput=False),
        "wk": nc.declare_dram_parameter("wk", [D, DQ], BF, isOutput=False),
        "wv": nc.declare_dram_parameter("wv", [D, DQ], BF, isOutput=False),
        "bv": nc.declare_dram_parameter("bv", [DQ], FP, isOutput=False),
        "wo": nc.declare_dram_parameter("wo", [D, DQ], BF, isOutput=False),
        "bo": nc.declare_dram_parameter("bo", [DQ], FP, isOutput=False),
        "out": nc.declare_dram_parameter("out", [S, DQ], BF, isOutput=True),
    }
    if dbg:
        io["dbg_y"] = nc.declare_dram_parameter(
            "dbg_y", [DQ, S], BF, isOutput=True)
        io["dbg_yg"] = nc.declare_dram_parameter(
            "dbg_yg", [2 * DQ, S], BF, isOutput=True)
    io = {k: (v[:] if not isinstance(v, bass.AP) else v) for k, v in io.items()}
    with tile.TileContext(nc) as tc:
        emit_mha(nc, tc, io, S, dbg=dbg)
    nc.finalize()
    return nc


def shard_inputs(x, Wq, bq, Wk, bk, Wv, bv, Wo, bo):
    """Full inputs -> per-core in_maps. Matmul operands cast to bf16; x is
    transposed on the host (input prep for the d-major device layout)."""
    BFNP = ml_dtypes.bfloat16
    f32 = lambda a: np.ascontiguousarray(np.asarray(a), dtype=np.float32)
    bf = lambda a: np.ascontiguousarray(np.asarray(a, dtype=np.float32)
                                        .astype(BFNP))
    x = np.asarray(x, dtype=np.float32).astype(BFNP)
    S = x.shape[1]
    NB = S // 512
    DKT = D // 128
    # x[b].T [D, S] -> chunk-major [(DKT*NB)*128, 512] (contiguous DMA src)
    xts = [np.ascontiguousarray(
        x[b].T.reshape(DKT, 128, NB, 512).transpose(0, 2, 1, 3)
        .reshape(DKT * NB * 128, 512)) for b in range(4)]
    Wq, Wk, Wv, Wo = bf(Wq), bf(Wk), bf(Wv), bf(Wo)
    bq, bk, bv, bo = f32(bq), f32(bk), f32(bv), f32(bo)
    in_maps = []
    for c in range(NCORES):
        b, g = divmod(c, 2)
        sl = slice(g * DQ, (g + 1) * DQ)
        bqk = np.empty((128, 8), np.float32)
        for m in range(4):
            bqk[:, m] = bq[sl][m * 128:(m + 1) * 128]
            bqk[:, 4 + m] = bk[sl][m * 128:(m + 1) * 128]
        in_maps.append({
            "xt": xts[b],
            "wq": np.ascontiguousarray(Wq[:, sl]), "bqk": bqk,
            "wk": np.ascontiguousarray(Wk[:, sl]),
            "wv": np.ascontiguousarray(Wv[:, sl]), "bv": bv[sl].copy(),
            "wo": np.ascontiguousarray(Wo[:, sl]), "bo": bo[sl].copy(),
        })
    return in_maps


_CACHE = {}


def _get_program(S=2048):
    if S not in _CACHE:
        _CACHE[S] = build_program(S)
    return _CACHE[S]


def kernel(x, Wq, bq, Wk, bk, Wv, bv, Wo, bo):
    nc = _get_program(2048)
    in_maps = shard_inputs(x, Wq, bq, Wk, bk, Wv, bv, Wo, bo)
    res = run_bass_kernel_spmd(nc, in_maps, list(range(NCORES))).results
    S = 2048
    out = np.empty((4, S, D), dtype=np.float32)
    for c in range(NCORES):
        b, g = divmod(c, 2)
        out[b, :, g * DQ:(g + 1) * DQ] = np.asarray(
            res[c]["out"]).astype(np.float32)
    return out


# revision 53
# speedup vs baseline: 1.0051x; 1.0051x over previous
"""Trainium2 Bass kernel: multi-head attention (b=4, s=2048, d_model=1024, h=16).

Sharding over 8 NeuronCores: 2-D (batch x head-half).
  core c -> batch c//2, head group c%2 (8 of 16 heads, qkv dims 512*g..512*g+512).
Per core: QKV column-parallel, per-head attention (scores computed transposed,
softmax sums via a ones-column appended to V in the PV matmul, max-subtraction
skipped -- scores are O(5) so exp is safe), then a pairwise AllGather of the
normalized per-head outputs and a column-parallel output projection.

All matmul operands are bf16 (fp32 PSUM accumulation); fp32 matmul on trn2
costs two array passes, bf16 one. The host pre-transposes x to x^T [D, S]
and casts to bf16 (input prep), so no on-device transpose is needed.

QKV projection (per head-pair) and attention are emitted interleaved so the
scalar-engine exp stream (the critical resource) starts as early as possible
and PE fills its gaps with the next head-pair's projections. The AllGather
is split per head-pair so all but the last overlap attention.

v2 changes (from trace analysis of the 421us baseline):
 - PE warm-up matmul stream at t=0 so the HAM clock gate reaches 2.4 GHz
   before real matmuls arrive (PE ran at 1.2 GHz until 35us).
 - Consolidated input loads into one fat DMA per tensor (DMA issue on the
   sync queue costs ~590ns each; 24 thin descriptors gated the first
   matmul at 17.9us).
 - Softmax extract/normalize rebuilt: acc PSUM released after two fp32
   copies, reciprocal via reciprocal_approx_fast (5x cheaper than the
   3.3us RECIPROCAL that stalled PV between units), fp32 broadcast (no
   bf16 cast hop), normalize fused into the evacuation multiply. Per-unit
   small DMAs moved to the gpsimd queue.
 - Tail: hp2/hp3 gathered in 512-col chunks, staging DMAs are 2 fat
   descriptors, and the final Wo phase-B is emitted per 512-token quarter
   right after its staging flush so the endgame pipelines instead of
   serializing (~40us tail before).

Host assembly: out[b] = concat(core 2b cols 0:512, core 2b+1 cols 512:1024).

Self-contained: hardcodes all shapes; builds/compiles once per process.
"""

from contextlib import ExitStack

import ml_dtypes
import numpy as np

import concourse.bass as bass
import concourse.mybir as mybir
import concourse.tile as tile
from concourse import bacc
from concourse.bass_utils import run_bass_kernel_spmd

FP = mybir.dt.float32
FR = mybir.dt.float32r
BF = mybir.dt.bfloat16
AFT = mybir.ActivationFunctionType
ts = bass.ts

NCORES = 8
D = 1024           # d_model
HD = 64            # head dim
HPC = 8            # heads per core
DQ = HPC * HD      # per-core qkv width = 512
SCALE = 1.0 / np.sqrt(HD)
N_WARMUP = 8       # PE warm-up matmuls: ~4.5us cold, bridges to data-ready


def emit_mha(nc, tc, io, S, dbg=False):
    """Emit the per-core MHA program. io: dict of DRAM APs."""
    NHP = HPC // 2       # head pairs = 4
    KT = S // 128        # sk tiles
    SQB = S // 512       # sq blocks of 512
    DKT = D // 128       # d_in tiles = 8
    MQ = DQ // 128       # qkv dout tiles = 4
    TT = S // 128        # token tiles
    NB = S // 512        # token blocks of 512

    xt_in, wq_in, bqk_in, wk_in, wv_in, bv_in, wo_in, bo_in, out_ext = (
        io["xt"], io["wq"], io["bqk"], io["wk"], io["wv"], io["bv"],
        io["wo"], io["bo"], io["out"])

    with ExitStack() as ctx:
        const_pool = ctx.enter_context(tc.tile_pool(name="const", bufs=1))
        dram_pool = ctx.enter_context(tc.tile_pool(name="dram", bufs=1, space="DRAM"))
        # one shared PSUM budget: mm 2 + scores 4 + accA 1 + accB 1 = 8 banks
        mm_psum = ctx.enter_context(
            tc.tile_pool(name="mmps", bufs=2, space="PSUM"))
        sc_psum = ctx.enter_context(
            tc.tile_pool(name="scps", bufs=2, space="PSUM"))
        ac_psum = ctx.enter_context(
            tc.tile_pool(name="acps", bufs=1, space="PSUM"))

        # ---- PE warm-up: HAM-unthrottle the tensor engine while the input
        # DMAs are in flight. Reuses the mm psum tag (no extra banks). ----
        warm = const_pool.tile([128, 512], BF, tag="warm", name="warm")
        nc.gpsimd.memset(warm[:], 1.0)
        for _ in range(N_WARMUP):
            wp = mm_psum.tile([128, 512], FP, tag="mm", name="mm")
            nc.tensor.matmul(wp[:], lhsT=warm[:, 0:128],
                             rhs=warm[:], start=True, stop=True)

        # biases for q/k, host-packed [128, 2*MQ]: col m = bq tile m, MQ+m = bk
        bias_qk = const_pool.tile([128, 2 * MQ], FP, tag="bqk", name="bqk")
        nc.sync.dma_start(bias_qk[:], bqk_in[:, :])



        # bv / bo broadcast tiles; bo twice along the free dim since phase A
        # adds it over [128, 1024] psum pairs
        bv_bc = const_pool.tile([128, DQ], FP, tag="bvbc", name="bvbc")
        bo_bc2 = const_pool.tile([128, 2 * DQ], FP, tag="bobc", name="bobc")
        with tc.tile_pool(name="btmpp", bufs=1) as btmp_pool:
            btmp = btmp_pool.tile([128, DQ], FP, tag="btmp", name="btmp")
            nc.sync.dma_start(
                btmp[0:1, :], bv_in[:].rearrange("(one f) -> one f", one=1))
            nc.gpsimd.partition_broadcast(bv_bc[:], btmp[0:1, :])
            btmp2 = btmp_pool.tile([128, 2 * DQ], FP, tag="btmp2",
                                   name="btmp2")
            for h in range(2):
                nc.sync.dma_start(
                    btmp2[0:1, ts(h, DQ)],
                    bo_in[:].rearrange("(one f) -> one f", one=1))
            nc.gpsimd.partition_broadcast(bo_bc2[:], btmp2[0:1, :])

        # DRAM bounce + AllGather in/out (bf16); collective operands must be
        # contiguous. Early head-pairs gather per seq-half; the last two
        # gather in 512-col chunks so the endgame only ever waits on 512
        # columns and the final Wo accumulation pipelines per quarter.
        def seq_chunks(hp):
            if hp < 2:
                return [(1, 0, S // 2), (3, S // 2, S // 2)]
            if hp == 2:
                return [(1, 0, S // 2), (2, S // 2, 512), (3, S // 2 + 512, 512)]
            return [(0, 0, 512), (1, 512, 512), (2, 1024, 512), (3, 1536, 512)]

        y_bnc = {}
        y_gath = {}
        for hp in range(NHP):
            for (sd, c0, w) in seq_chunks(hp):
                y_bnc[hp, c0] = dram_pool.tile(
                    [128, w], BF, tag=f"ybounce{hp}_{c0}",
                    name=f"ybounce{hp}_{c0}")
                y_gath[hp, c0] = dram_pool.tile(
                    [256, w], BF, tag=f"ygather{hp}_{c0}",
                    name=f"ygather{hp}_{c0}")

        with ExitStack() as phase12:
            qkv_pool = phase12.enter_context(tc.tile_pool(name="qkv", bufs=1))
            yt_pool = phase12.enter_context(tc.tile_pool(name="yt", bufs=1))
            exp_pool = phase12.enter_context(tc.tile_pool(name="exp", bufs=6))
            stage_pool = phase12.enter_context(tc.tile_pool(name="stage", bufs=2))

            # q^T / k^T, d-major: tile hp holds heads 2hp (parts 0-63), 2hp+1
            qT = [qkv_pool.tile([128, S], BF, tag=f"qT{m}", name=f"qT{m}")
                  for m in range(MQ)]
            kT = [qkv_pool.tile([128, S], BF, tag=f"kT{m}", name=f"kT{m}")
                  for m in range(MQ)]
            # v natural [tok, dout] with a ones column per head
            v_ones = [qkv_pool.tile([128, HPC * (HD + 1)], BF, tag=f"v{t}",
                                    name=f"v{t}")
                      for t in range(TT)]

            # Wo + gathered-y staging (loaded during attention so the
            # final projection phase starts compute immediately)
            yg = [qkv_pool.tile([128, S], BF, tag=f"yg{k}", name=f"yg{k}")
                  for k in range(2 * MQ)]

            def stage_yg(hp, c0, w, eng=None):
                # copy gathered chunk into the staging tiles (2 fat DMAs).
                # In-loop flushes (wait-free, gather long done) ride the
                # fast sync DGE. The post-loop, gather-waiting flushes are
                # issued from the scalar queue instead: sync-queue DMAs
                # share round-robin completion semaphores, so a
                # gather-waiting stage DMA there poisons the wait of the
                # next collective trigger (observed +10us on the last
                # gather).
                eng = eng or nc.sync
                gath = y_gath[hp, c0]
                eng.dma_start(yg[hp][:, c0:c0 + w], gath[0:128, :])
                eng.dma_start(yg[MQ + hp][:, c0:c0 + w], gath[128:256, :])

            with ExitStack() as phase01:
                # ---- load x^T (pre-transposed on host) and weights; one
                # fat DMA per tensor (issue rate, not bandwidth, gated the
                # kernel head) ----
                xtw_pool = phase01.enter_context(tc.tile_pool(name="xtw", bufs=1))
                xTall = xtw_pool.tile([128, DKT * S], BF, tag="xTall",
                                      name="xTall")
                xT3 = xTall[:].rearrange("p (d s) -> p d s", s=S)
                xT4 = xTall[:].rearrange("p (d nb s) -> p d nb s", nb=NB,
                                         s=512)
                xt_src = xt_in.rearrange("(k nb p) s -> p k nb s", k=DKT,
                                         nb=NB, p=128)

                def xTs(k, sl):
                    return xT3[:, k, sl]

                def load_x_nb(nb, k0=0, k1=DKT):
                    nc.sync.dma_start(xT4[:, k0:k1, nb, :],
                                      xt_src[:, k0:k1, nb, :])

                # weights as single tiles, k-major free layout:
                # w_all[p, k*DQ + c] = W[k*128 + p, c]. All bulk loads ride
                # the sync DGE (the only ~155GB/s path; scalar/gpsimd DGEs
                # measured far slower), ordered by first consumption: the
                # m=0 column chunks of wq/wk land first so head-pair 0's
                # projections (and the exp stream) start ~14us in.
                def walloc(nm):
                    return xtw_pool.tile([128, DKT * DQ], BF, tag=nm,
                                         name=nm)

                def load_w_cols(t, w_in, c0, c1):
                    nc.sync.dma_start(
                        t[:].rearrange("p (k c) -> p k c", c=DQ)[:, :, c0:c1],
                        w_in.rearrange("(k p) c -> p k c", p=128)[:, :, c0:c1])

                wq_all, wk_all, wv_all, wo_all = (
                    walloc("wqa"), walloc("wka"), walloc("wva"), walloc("woa"))
                load_x_nb(0, 0, 4)
                load_w_cols(wq_all, wq_in, 0, 128)
                load_w_cols(wk_all, wk_in, 0, 128)
                load_x_nb(0, 4, 8)
                load_w_cols(wv_all, wv_in, 0, DQ)
                load_x_nb(1)
                load_w_cols(wq_all, wq_in, 128, DQ)
                load_w_cols(wk_all, wk_in, 128, DQ)
                load_w_cols(wo_all, wo_in, 0, DQ)
                load_x_nb(2)
                load_x_nb(3)

                def emit_qk_group(m, g):
                    # one q/k projection psum group for head-pair m;
                    # g//NB selects q vs k, g%NB the token block. Evac on
                    # DVE (keeps the scalar engine free for the exp stream).
                    w_all, bcol, dstT = ((wq_all, 0, qT), (wk_all, 1, kT))[g // NB]
                    nb = g % NB
                    ps = mm_psum.tile([128, 512], FP, tag="mm", name="mm")
                    for k in range(DKT):
                        nc.tensor.matmul(
                            ps[:], lhsT=w_all[:, k * DQ + m * 128:
                                              k * DQ + (m + 1) * 128],
                            rhs=xTs(k, ts(nb, 512)),
                            start=(k == 0), stop=(k == DKT - 1))
                    col = bcol * MQ + m
                    nc.vector.tensor_scalar_add(
                        dstT[m][:, ts(nb, 512)], ps[:],
                        bias_qk[:, col:col + 1])

                def emit_qk_pair(m, ga, gb):
                    # two same-family projection groups emitted with the
                    # k-loop interleaved: consecutive matmuls share their
                    # stationary weight block, letting the PE skip/overlap
                    # the second LDWEIGHTS.
                    w_all, bcol, dstT = ((wq_all, 0, qT), (wk_all, 1, kT))[ga // NB]
                    assert gb // NB == ga // NB
                    nba, nbb = ga % NB, gb % NB
                    psa = mm_psum.tile([128, 512], FP, tag="mm", name="mm")
                    psb = mm_psum.tile([128, 512], FP, tag="mm", name="mm")
                    for k in range(DKT):
                        wsl = w_all[:, k * DQ + m * 128: k * DQ + (m + 1) * 128]
                        nc.tensor.matmul(
                            psa[:], lhsT=wsl, rhs=xTs(k, ts(nba, 512)),
                            start=(k == 0), stop=(k == DKT - 1),
                            skip_group_check=True)
                        nc.tensor.matmul(
                            psb[:], lhsT=wsl, rhs=xTs(k, ts(nbb, 512)),
                            start=(k == 0), stop=(k == DKT - 1),
                            skip_group_check=True)
                    col = bcol * MQ + m
                    nc.vector.tensor_scalar_add(
                        dstT[m][:, ts(nba, 512)], psa[:],
                        bias_qk[:, col:col + 1])
                    nc.vector.tensor_scalar_add(
                        dstT[m][:, ts(nbb, 512)], psb[:],
                        bias_qk[:, col:col + 1])

                def emit_v(t0, t1):
                    for ti in range(t0, t1):
                        ps = mm_psum.tile([128, DQ], FP, tag="mm", name="mm")
                        for k in range(DKT):
                            nc.tensor.matmul(
                                ps[:], lhsT=xTs(k, ts(ti, 128)),
                                rhs=wv_all[:, ts(k, DQ)],
                                start=(k == 0), stop=(k == DKT - 1))
                        vt3 = v_ones[ti][:].rearrange("p (h u) -> p h u",
                                                      u=HD + 1)
                        nc.vector.tensor_add(
                            vt3[:, :, 0:HD],
                            ps[:].rearrange("p (h u) -> p h u", u=HD),
                            bv_bc[:].rearrange("p (h u) -> p h u", u=HD))
                        nc.gpsimd.memset(vt3[:, :, HD:HD + 1], 1.0)

                # ---- per head-pair: qk projection then attention; hp0's
                # q/k groups interleaved into the first unit so the exp
                # stream starts asap; gathered-y staging DMAs deferred 2
                # units so they never wait on an in-flight collective ----
                pending_stage = []

                def flush_stage(unit_now):
                    while pending_stage and pending_stage[0][3] <= unit_now - 2:
                        rhp, rc0, rw, _ = pending_stage.pop(0)
                        stage_yg(rhp, rc0, rw)

                for hp in range(NHP):
                    if hp == 0:
                        emit_qk_group(0, 0)       # q nb0
                        emit_qk_group(0, NB)      # k nb0

                    # y^T for this head-pair (d-major, normalized in place
                    # per unit); ring of 2 so hp+1 overlaps hp's shipping
                    yTh = yt_pool.tile([128, S], BF, tag="yT", bufs=2,
                                       name="yT")
                    hA, hB = 2 * hp, 2 * hp + 1
                    for sqb in range(SQB):
                        if hp == 0 and sqb > 0:
                            emit_qk_group(0, sqb)  # q nb<sqb>
                        sq = ts(sqb, 512)
                        accA = ac_psum.tile([HD + 1, 512], FP, tag="accA",
                                            name="accA")
                        accB = ac_psum.tile([HD + 1, 512], FP, tag="accB",
                                            name="accB")
                        for k in range(KT):
                            if hp == 0 and sqb == 0 and k % 4 == 0 and k > 0:
                                emit_qk_group(0, NB + k // 4)  # k nb<k//4>
                            sk = ts(k, 128)
                            ps = sc_psum.tile([128, 1024], FP, tag="sc", name="sc")
                            # scores^T [sk, sq] for both heads (row-tiled pair)
                            nc.tensor.matmul(
                                ps[:, 0:512], lhsT=kT[hp][0:64, sk],
                                rhs=qT[hp][0:64, sq], start=True, stop=True)
                            nc.tensor.matmul(
                                ps[:, 512:1024], lhsT=kT[hp][64:128, sk],
                                rhs=qT[hp][64:128, sq], start=True, stop=True)
                            et = exp_pool.tile([128, 1024], BF, tag="exp",
                                               name="exp")
                            nc.scalar.activation(et[:], ps[:], AFT.Exp,
                                                 scale=SCALE)
                            if hp == 0 and sqb == 0:
                                # produce v[k] just in time for its attnv
                                emit_v(k, k + 1)
                            # y^T accumulation: lhsT = [v_h | 1]
                            nc.tensor.matmul(
                                accA[:], lhsT=v_ones[k][:, hA * 65:hA * 65 + 65],
                                rhs=et[:, 0:512],
                                start=(k == 0), stop=(k == KT - 1),
                                skip_group_check=True)
                            nc.tensor.matmul(
                                accB[:], lhsT=v_ones[k][:, hB * 65:hB * 65 + 65],
                                rhs=et[:, 512:1024],
                                start=(k == 0), stop=(k == KT - 1),
                                skip_group_check=True)
                        # ---- extract + normalize ----
                        # Two fp32 copies release the acc PSUM fast; the
                        # sums rows hop to partition 0 (sync DMA, short
                        # wait), feed reciprocal_approx_fast (sums are
                        # O(1e2..1e5), far from its edge cases), an fp32
                        # partition broadcast, and a normalize multiply
                        # fused with the evacuation. Queue placement breaks
                        # the cross-queue cycle seen in traces: sync holds
                        # only short-wait hops + the ship, while the hop
                        # that waits on the multiply rides gpsimd ahead of
                        # the collective trigger it feeds.
                        stA = stage_pool.tile([128, 512], FP, tag="stA",
                                              name="stA")
                        stB = stage_pool.tile([128, 512], FP, tag="stB",
                                              name="stB")
                        nc.vector.tensor_copy(stA[0:65, :], accA[0:65, :])
                        nc.vector.tensor_copy(stB[0:65, :], accB[0:65, :])
                        bbA = stage_pool.tile([1, 512], FP, tag="bbA",
                                              name="bbA")
                        bbB = stage_pool.tile([1, 512], FP, tag="bbB",
                                              name="bbB")
                        nc.sync.dma_start(bbA[0:1, :], stA[64:65, :])
                        nc.sync.dma_start(bbB[0:1, :], stB[64:65, :])
                        nc.vector.reciprocal_approx_fast(bbA[0:1, :],
                                                         bbA[0:1, :])
                        nc.vector.reciprocal_approx_fast(bbB[0:1, :],
                                                         bbB[0:1, :])
                        rbA = stage_pool.tile([128, 512], FP, tag="rbA",
                                              name="rbA")
                        rbB = stage_pool.tile([128, 512], FP, tag="rbB",
                                              name="rbB")
                        nc.gpsimd.partition_broadcast(rbA[0:64, :], bbA[0:1, :])
                        nc.gpsimd.partition_broadcast(rbB[0:64, :], bbB[0:1, :])
                        nc.vector.tensor_mul(yTh[0:64, sq], stA[0:64, :],
                                             rbA[0:64, :])
                        stB16 = stage_pool.tile([128, 512], BF, tag="stB16",
                                                name="stB16")
                        nc.vector.tensor_mul(stB16[0:64, :], stB[0:64, :],
                                             rbB[0:64, :])
                        nc.gpsimd.dma_start(yTh[64:128, sq], stB16[0:64, :])
                        # drip the next head-pair's projections between
                        # units (paired: both groups share weight blocks).
                        # K-family first: the next head-pair's first scores
                        # need ALL of kT (every sk tile) but only one sq
                        # block of qT, and k-last caused a ~10us exp stall
                        # at each head-pair transition.
                        if hp + 1 < NHP:
                            ga = (2 * sqb + NB) % (2 * NB)
                            emit_qk_pair(hp + 1, ga, ga + 1)
                        # ship and AllGather each completed seq chunk of this
                        # hp's y^T; defer the gathered-y staging DMAs
                        for (sd, c0, w) in seq_chunks(hp):
                            if sd != sqb:
                                continue
                            nc.sync.dma_start(y_bnc[hp, c0][:, :],
                                              yTh[:, c0:c0 + w])
                            nc.gpsimd.collective_compute(
                                "AllGather", mybir.AluOpType.bypass,
                                replica_groups=[[0, 1], [2, 3],
                                                [4, 5], [6, 7]],
                                ins=[y_bnc[hp, c0][:, :]],
                                outs=[y_gath[hp, c0][:, :]])
                            pending_stage.append(
                                (hp, c0, w, hp * SQB + sqb))
                        flush_stage(hp * SQB + sqb)
                # ---- Phase A of the output projection: per token tile,
                # accumulate head-pair-0..2 contributions (6 of 8 k2 blocks,
                # staged well before attention ends) + bias into a bf16
                # partial parked in a dead qT/kT tile. Emitted AFTER the hp
                # loop so its priority is lowest: the scheduler runs these
                # matmuls in PE-idle slots of hp3's exp-paced units. Only
                # tiles 0-11 (72 matmuls): that is what the real slack of
                # the exp-paced hp3 units holds. With all 16 tiles, the
                # overflow group lands after the whole attention stream and
                # its DVE add -- statically ordered mid-stream by the
                # scheduler's optimistic PE model -- head-of-line-blocks
                # the unit-14/15 extract ops for ~10us (semaphores are
                # counters over the static per-engine order). Tiles 12-15
                # run as full 8-band groups in phase B instead. ----
                for t0 in range(0, TT, 2):
                    pos = [mm_psum.tile([128, DQ], FP, tag="mm", name="mm")
                           for _ in range(2)]
                    for j, k2 in enumerate((0, 1, 2, 4, 5, 6)):
                        for po, t in zip(pos, (t0, t0 + 1)):
                            nc.tensor.matmul(
                                po[:], lhsT=yg[k2][:, ts(t, 128)],
                                rhs=wo_all[:, ts(k2, DQ)],
                                start=(j == 0), stop=(j == 5),
                                skip_group_check=True)
                    for po, t in zip(pos, (t0, t0 + 1)):
                        slot = (qT[:3] + [kT[0]])[t // 4]
                        nc.vector.tensor_add(
                            slot[:, ts(t % 4, 512)], po[:],
                            bo_bc2[:, 0:DQ])

                # ---- Phase B: add the head-pair-3 contribution and store,
                # pipelined per 512-token quarter behind its staging. ----
                with tc.tile_pool(name="outp", bufs=3) as out_pool:
                    for q in range(SQB):
                        while pending_stage and (
                                pending_stage[0][0] < NHP - 1
                                or pending_stage[0][1] <= 512 * q):
                            rhp, rc0, rw, _ = pending_stage.pop(0)
                            stage_yg(rhp, rc0, rw)
                        for t0 in (4 * q, 4 * q + 2):
                            pos = [mm_psum.tile([128, DQ], FP, tag="mm",
                                                name="mm")
                                   for _ in range(2)]
                            k2s = (NHP - 1, MQ + NHP - 1)
                            for j, k2 in enumerate(k2s):
                                for po, t in zip(pos, (t0, t0 + 1)):
                                    nc.tensor.matmul(
                                        po[:], lhsT=yg[k2][:, ts(t, 128)],
                                        rhs=wo_all[:, ts(k2, DQ)],
                                        start=(j == 0),
                                        stop=(j == len(k2s) - 1),
                                        skip_group_check=True)
                            for po, t in zip(pos, (t0, t0 + 1)):
                                ot = out_pool.tile([128, DQ], BF, tag="ot",
                                                   name="ot")
                                slot = (qT[:3] + [kT[0]])[t // 4]
                                nc.vector.tensor_add(
                                    ot[:], po[:], slot[:, ts(t % 4, 512)])
                                # bf16 stores (host upcasts; halves the
                                # store bytes) on the scalar queue (idle
                                # once the exp stream ends); they must not
                                # block sync (ships) or gpsimd (triggers +
                                # staging). The last quarter's stores ride
                                # the fast sync DGE: nothing queues behind
                                # them there.
                                eng = nc.sync if q == SQB - 1 else nc.scalar
                                eng.dma_start(out_ext[ts(t, 128), :], ot[:])


def build_program(S=2048, dbg=False):
    nc = bacc.Bacc(
        "TRN2",
        target_bir_lowering=False,
        debug=False,
        enable_asserts=True,
        num_devices=NCORES,
    )
    NB = S // 512
    DKT = D // 128
    io = {
        "xt": nc.declare_dram_parameter("xt", [DKT * NB * 128, 512], BF,
                                        isOutput=False),
        "wq": nc.declare_dram_parameter("wq", [D, DQ], BF, isOutput=False),
        "bqk": nc.declare_dram_parameter("bqk", [128, 8], FP, isOutput=False),
        "wk": nc.declare_dram_parameter("wk", [D, DQ], BF, isOutput=False),
        "wv": nc.declare_dram_parameter("wv", [D, DQ], BF, isOutput=False),
        "bv": nc.declare_dram_parameter("bv", [DQ], FP, isOutput=False),
        "wo": nc.declare_dram_parameter("wo", [D, DQ], BF, isOutput=False),
        "bo": nc.declare_dram_parameter("bo", [DQ], FP, isOutput=False),
        "out": nc.declare_dram_parameter("out", [S, DQ], BF, isOutput=True),
    }
    if dbg:
        io["dbg_y"] = nc.declare_dram_parameter(
            "dbg_y", [DQ, S], BF, isOutput=True)
        io["dbg_yg"] = nc.declare_dram_parameter(
            "dbg_yg", [2 * DQ, S], BF, isOutput=True)
    io = {k: (v[:] if not isinstance(v, bass.AP) else v) for k, v in io.items()}
    with tile.TileContext(nc) as tc:
        emit_mha(nc, tc, io, S, dbg=dbg)
    nc.finalize()
    return nc


def shard_inputs(x, Wq, bq, Wk, bk, Wv, bv, Wo, bo):
    """Full inputs -> per-core in_maps. Matmul operands cast to bf16; x is
    transposed on the host (input prep for the d-major device layout)."""
    BFNP = ml_dtypes.bfloat16
    f32 = lambda a: np.ascontiguousarray(np.asarray(a), dtype=np.float32)
    bf = lambda a: np.ascontiguousarray(np.asarray(a, dtype=np.float32)
                                        .astype(BFNP))
    x = np.asarray(x, dtype=np.float32).astype(BFNP)
    S = x.shape[1]
    NB = S // 512
    DKT = D // 128
    # x[b].T [D, S] -> chunk-major [(DKT*NB)*128, 512] (contiguous DMA src)
    xts = [np.ascontiguousarray(
        x[b].T.reshape(DKT, 128, NB, 512).transpose(0, 2, 1, 3)
        .reshape(DKT * NB * 128, 512)) for b in range(4)]
    Wq, Wk, Wv, Wo = bf(Wq), bf(Wk), bf(Wv), bf(Wo)
    bq, bk, bv, bo = f32(bq), f32(bk), f32(bv), f32(bo)
    in_maps = []
    for c in range(NCORES):
        b, g = divmod(c, 2)
        sl = slice(g * DQ, (g + 1) * DQ)
        bqk = np.empty((128, 8), np.float32)
        for m in range(4):
            bqk[:, m] = bq[sl][m * 128:(m + 1) * 128]
            bqk[:, 4 + m] = bk[sl][m * 128:(m + 1) * 128]
        in_maps.append({
            "xt": xts[b],
            "wq": np.ascontiguousarray(Wq[:, sl]), "bqk": bqk,
            "wk": np.ascontiguousarray(Wk[:, sl]),
            "wv": np.ascontiguousarray(Wv[:, sl]), "bv": bv[sl].copy(),
            "wo": np.ascontiguousarray(Wo[:, sl]), "bo": bo[sl].copy(),
        })
    return in_maps


_CACHE = {}


def _get_program(S=2048):
    if S not in _CACHE:
        _CACHE[S] = build_program(S)
    return _CACHE[S]


def kernel(x, Wq, bq, Wk, bk, Wv, bv, Wo, bo):
    nc = _get_program(2048)
    in_maps = shard_inputs(x, Wq, bq, Wk, bk, Wv, bv, Wo, bo)
    res = run_bass_kernel_spmd(nc, in_maps, list(range(NCORES))).results
    S = 2048
    out = np.empty((4, S, D), dtype=np.float32)
    for c in range(NCORES):
        b, g = divmod(c, 2)
        out[b, :, g * DQ:(g + 1) * DQ] = np.asarray(
            res[c]["out"]).astype(np.float32)
    return out
